# revision 5
# baseline (speedup 1.0000x reference)
"""Linear-memory attention (scatter_memory) Trainium2 Bass kernel.

Problem (hardcoded shapes):
  B=2, S=4096, HID=4096, H=32 query heads, HKV=8 kv heads, D=128, G=4.
  reference:
    q = hs @ Wq.T + bq ; k = hs @ Wk.T + bk ; v = hs @ Wv.T + bv
    sig = elu(.)+1 ; q_sel = sig_q[:, :, ::G]          # only 8 of 32 q heads used
    retrieved = (q_sel @ M) / (q_sel . z + eps)
    output    = repeat(retrieved, G) @ Wo.T
    M_new     = M + sig_k^T v (summed over b,s) ; z_new = z + sum sig_k

Algebraic reductions baked in:
  * Only query heads 0,4,...,28 are used -> project just those 8 heads (4x less Q work).
  * repeat_interleave + Wo folds into Wo_eff[:, hkv] = sum_g Wo[:, (hkv*G+g)*D:(...)]
    (4x less o_proj work).

Sharding: pure data-parallel over the 8192 flattened (b,s) tokens - 1024
tokens per core, weights replicated, no collectives. Each core returns its
disjoint output rows plus partial dM/dz; the tiny (0.5 MB) M/z reductions
are summed on host.

Device kernel per core (all GEMMs in float32r, ~1e-4 rel err):
  1. PE-transpose hs rows -> hsT [hid, tok] resident in SBUF (16 MB).
  2. QKV: stationary = W^T tiles (streamed once), moving = hsT -> psum [out,
     tok]; epilogue applies elu+1 (relu + exp(x-relu(x))) and spills
     sig_qT/sig_kT/vT per head to DRAM scratch; dz = row-sum of sig_kT.
  3. Retrieve per kv head: numT = M_h^T-free matmul (M_h stationary, sig_qT
     moving), den = z_h . sig_qT, retrT = numT * (1/(den+eps)) broadcast.
  4. dM: re-transpose sig_kT/vT to token-major, accumulate sig_k^T v in psum.
  5. o_proj: stationary = retrT tiles, moving = Wo_eff^T -> output rows.
"""

import os
import sys

sys.path.insert(0, "/opt/trn_rl_repo")

import numpy as np

import concourse.bass as bass
import concourse.mybir as mybir
import concourse.tile as tile
from concourse import bacc, bass_utils
from concourse.masks import make_identity

F32 = mybir.dt.float32
F32R = mybir.dt.float32r
AF = mybir.ActivationFunctionType

B, S, HID = 2, 4096, 4096
H, HKV, D = 32, 8, 128
G = H // HKV
EPS = 1e-6
NCORES = 8
NTOK = (B * S) // NCORES      # 1024 tokens per core
TB = NTOK // 128              # 8 token blocks
KT = HID // 128               # 32 contraction tiles
MQKV = 24                     # 24 output tiles (8 q_sel + 8 k + 8 v heads)

_cached_nc = None

# Optional NTFF tracing (test harness sets BASS_KERNEL_TRACE=1 to get HW
# exec time); grading path stays trace-free and needs no profile hook.
TRACE = os.environ.get("BASS_KERNEL_TRACE", "0") == "1"
last_results = None
if TRACE:
    try:
        import antenv.axon_hooks as _ah
        from trn_agent_boot.trn_boot import _ntff_profile_via_ctypes

        if _ah.get_axon_ntff_profile_hook() is None:
            _ah.set_axon_ntff_profile_hook(
                _ntff_profile_via_ctypes("/opt/axon/libaxon_pjrt.so")
            )
        bass_utils.upload_artifacts = lambda tmpdir: f"local://{tmpdir}"
    except Exception:
        TRACE = False


def _build():
    nc = bacc.Bacc("TRN2", target_bir_lowering=False, debug=False,
                   num_devices=NCORES)

    hs_d = nc.dram_tensor("hs", [NTOK, HID], F32, kind="ExternalInput")
    wt_d = nc.dram_tensor("wt", [MQKV, 128, HID], F32R, kind="ExternalInput")
    wot_d = nc.dram_tensor("wot", [4, 128, 8 * NTOK], F32R, kind="ExternalInput")
    m_d = nc.dram_tensor("m_in", [128, HKV * D], F32R, kind="ExternalInput")
    z_d = nc.dram_tensor("z_in", [128, HKV], F32R, kind="ExternalInput")
    bias_d = nc.dram_tensor("bias", [128, MQKV], F32, kind="ExternalInput")

    out_d = nc.dram_tensor("out", [NTOK, HID], F32, kind="ExternalOutput")
    dm_d = nc.dram_tensor("dm", [128, HKV * D], F32, kind="ExternalOutput")
    dz_d = nc.dram_tensor("dz", [128, 16], F32, kind="ExternalOutput")

    with tile.TileContext(nc) as tc:
        with (
            tc.tile_pool(name="const", bufs=1) as constp,
            tc.tile_pool(name="scr", bufs=1, space="DRAM") as scrp,
        ):
            ident = constp.tile([128, 128], F32)
            make_identity(nc, ident[:])
            bias_sb = constp.tile([128, MQKV], F32)
            nc.sync.dma_start(bias_sb[:], bias_d[:, :])
            z_sb = constp.tile([128, HKV], F32R)
            nc.sync.dma_start(z_sb[:], z_d[:, :])
            dz_sb = constp.tile([128, 16], F32)
            eps_sb = constp.tile([128, 1], F32)
            nc.gpsimd.memset(eps_sb[:], EPS)

            scr_q = scrp.tile([HKV, 128, NTOK], F32R, tag="sq")
            scr_k = scrp.tile([HKV, 128, NTOK], F32R, tag="sk")
            scr_v = scrp.tile([HKV, 128, NTOK], F32R, tag="sv")

            # ---- stage 1+2: hs transpose, then QKV projections ----
            with tc.tile_pool(name="hsT", bufs=1) as hstp:
                hsT = hstp.tile([128, KT * NTOK], F32R)
                hsT3 = hsT[:].rearrange("p (k t) -> p k t", k=KT)

                with (
                    tc.tile_pool(name="hstage", bufs=2) as hstage,
                    tc.tile_pool(name="tpp", bufs=4, space="PSUM") as tpp,
                ):
                    for tb in range(TB):
                        hrow = hstage.tile([128, HID], F32)
                        nc.sync.dma_start(
                            hrow[:], hs_d[tb * 128 : (tb + 1) * 128, :]
                        )
                        for kg in range(KT // 4):
                            tp = tpp.tile([128, 512], F32, tag="tp")
                            for j in range(4):
                                kt = kg * 4 + j
                                nc.tensor.matmul(
                                    tp[:, j * 128 : (j + 1) * 128],
                                    hrow[:, kt * 128 : (kt + 1) * 128],
                                    ident[:],
                                    is_transpose=True,
                                    start=True,
                                    stop=True,
                                )
                            nc.scalar.activation(
                                hsT3[:, kg * 4 : kg * 4 + 4,
                                     tb * 128 : (tb + 1) * 128],
                                tp[:].rearrange("p (a b) -> p a b", a=4),
                                AF.Copy,
                            )

                with (
                    tc.tile_pool(name="wt", bufs=2) as wtp,
                    tc.tile_pool(name="sigt", bufs=3) as sigp,
                    tc.tile_pool(name="qkvps", bufs=4, space="PSUM") as qps,
                ):
                    for mt in range(MQKV):
                        wt_sb = wtp.tile([128, KT * 128], F32R, tag="w")
                        nc.sync.dma_start(wt_sb[:], wt_d[mt, :, :])
                        wt3 = wt_sb[:].rearrange("p (k j) -> p k j", k=KT)
                        bias_ap = bias_sb[:, mt : mt + 1]
                        for tt in range(2):
                            ps = qps.tile([128, 512], F32, tag="q")
                            for kt in range(KT):
                                nc.tensor.matmul(
                                    ps[:],
                                    wt3[:, kt, :],
                                    hsT3[:, kt, tt * 512 : (tt + 1) * 512],
                                    start=(kt == 0),
                                    stop=(kt == KT - 1),
                                )
                            sig = sigp.tile([128, 512], F32R, tag="sig")
                            if mt < 16:
                                # sig = elu(x+b)+1 = r + exp(x-r+b), r=relu(x+b)
                                r = sigp.tile([128, 512], F32, tag="r")
                                nc.scalar.activation(r[:], ps[:], AF.Relu,
                                                     bias=bias_ap)
                                dsub = sigp.tile([128, 512], F32, tag="d")
                                nc.vector.tensor_sub(dsub[:], ps[:], r[:])
                                ex = sigp.tile([128, 512], F32, tag="e")
                                nc.scalar.activation(ex[:], dsub[:], AF.Exp,
                                                     bias=bias_ap)
                                nc.vector.tensor_add(sig[:], r[:], ex[:])
                            else:
                                nc.vector.tensor_scalar_add(sig[:], ps[:],
                                                            bias_ap)
                            if mt < 8:
                                dst = scr_q[mt]
                            elif mt < 16:
                                dst = scr_k[mt - 8]
                            else:
                                dst = scr_v[mt - 16]
                            nc.sync.dma_start(
                                dst[:, tt * 512 : (tt + 1) * 512], sig[:]
                            )
                            if 8 <= mt < 16:
                                nc.vector.reduce_sum(
                                    dz_sb[:, (mt - 8) * 2 + tt :
                                          (mt - 8) * 2 + tt + 1],
                                    sig[:],
                                    axis=mybir.AxisListType.X,
                                )
                    nc.sync.dma_start(dz_d[:, :], dz_sb[:])

            # ---- stage 3: retrieve -> retrT ----
            with tc.tile_pool(name="retr", bufs=1) as retrp:
                retrT = retrp.tile([128, HKV * NTOK], F32R)
                retrT3 = retrT[:].rearrange("p (h t) -> p h t", h=HKV)

                with (
                    tc.tile_pool(name="qload", bufs=2) as qloadp,
                    tc.tile_pool(name="msb", bufs=1) as msbp,
                    tc.tile_pool(name="denp", bufs=2) as denp,
                    tc.tile_pool(name="retps", bufs=2, space="PSUM") as retps,
                    tc.tile_pool(name="denps", bufs=1, space="PSUM") as denps,
                ):
                    m_sb = msbp.tile([128, HKV * D], F32R)
                    nc.sync.dma_start(m_sb[:], m_d[:, :])
                    m_sb3 = m_sb[:].rearrange("p (h e) -> p h e", h=HKV)

                    for h in range(HKV):
                        qT = qloadp.tile([128, NTOK], F32R, tag="qT")
                        nc.sync.dma_start(qT[:], scr_q[h])
                        nps = retps.tile([128, NTOK], F32, tag="num")
                        dps = denps.tile([1, NTOK], F32, tag="den")
                        for tt in range(2):
                            sl = slice(tt * 512, (tt + 1) * 512)
                            nc.tensor.matmul(nps[:, sl], m_sb3[:, h, :],
                                             qT[:, sl], start=True, stop=True)
                            nc.tensor.matmul(dps[:, sl], z_sb[:, h : h + 1],
                                             qT[:, sl], start=True, stop=True)
                        den_s = denp.tile([1, NTOK], F32, tag="ds")
                        nc.vector.tensor_scalar_add(
                            den_s[:], dps[:], eps_sb[0:1, :]
                        )
                        recip = denp.tile([1, NTOK], F32, tag="rc")
                        nc.vector.reciprocal(recip[:], den_s[:])
                        bcast = denp.tile([128, NTOK], F32, tag="bc")
                        nc.gpsimd.partition_broadcast(bcast[:], recip[:])
                        nc.vector.tensor_mul(retrT3[:, h, :], nps[:], bcast[:])

                # ---- stage 4: dM accumulation ----
                with (
                    tc.tile_pool(name="kvload", bufs=4) as kvp,
                    tc.tile_pool(name="nat", bufs=2) as natp,
                    tc.tile_pool(name="dmps", bufs=1, space="PSUM") as dmpsp,
                    tc.tile_pool(name="tpp2", bufs=2, space="PSUM") as tpp2,
                    tc.tile_pool(name="dmout", bufs=1) as dmoutp,
                ):
                    dm_ps = dmpsp.tile([128, HKV * D], F32)
                    for h in range(HKV):
                        kT = kvp.tile([128, NTOK], F32R, tag="kT")
                        nc.sync.dma_start(kT[:], scr_k[h])
                        vT = kvp.tile([128, NTOK], F32R, tag="vT")
                        nc.sync.dma_start(vT[:], scr_v[h])
                        k_nat = natp.tile([128, TB * 128], F32R, tag="kn")
                        v_nat = natp.tile([128, TB * 128], F32R, tag="vn")
                        for src, dst in ((kT, k_nat), (vT, v_nat)):
                            dst3 = dst[:].rearrange("p (t j) -> p t j", t=TB)
                            for tg in range(TB // 4):
                                tp = tpp2.tile([128, 512], F32, tag="tp")
                                for j in range(4):
                                    tt = tg * 4 + j
                                    nc.tensor.matmul(
                                        tp[:, j * 128 : (j + 1) * 128],
                                        src[:, tt * 128 : (tt + 1) * 128]
                                        .bitcast(F32),
                                        ident[:],
                                        is_transpose=True,
                                        start=True,
                                        stop=True,
                                    )
                                nc.scalar.activation(
                                    dst3[:, tg * 4 : tg * 4 + 4, :],
                                    tp[:].rearrange("p (a b) -> p a b", a=4),
                                    AF.Copy,
                                )
                        k3 = k_nat[:].rearrange("p (t j) -> p t j", t=TB)
                        v3 = v_nat[:].rearrange("p (t j) -> p t j", t=TB)
                        for tt in range(TB):
                            nc.tensor.matmul(
                                dm_ps[:, h * D : (h + 1) * D],
                                k3[:, tt, :],
                                v3[:, tt, :],
                                start=(tt == 0),
                                stop=(tt == TB - 1),
                            )
                    dm_sb = dmoutp.tile([128, HKV * D], F32)
                    nc.scalar.activation(dm_sb[:], dm_ps[:], AF.Copy)
                    nc.sync.dma_start(dm_d[:, :], dm_sb[:])

                # ---- stage 5: o_proj ----
                with (
                    tc.tile_pool(name="wot", bufs=2) as wotp,
                    tc.tile_pool(name="osb", bufs=3) as osbp,
                    tc.tile_pool(name="ops", bufs=2, space="PSUM") as opsp,
                ):
                    for ng in range(4):
                        wot_sb = wotp.tile([128, 8 * NTOK], F32R, tag="wo")
                        nc.sync.dma_start(wot_sb[:], wot_d[ng, :, :])
                        wot3 = wot_sb[:].rearrange("p (h n) -> p h n", h=HKV)
                        for mt in range(TB):
                            po = opsp.tile([128, 1024], F32, tag="po")
                            for nn in range(2):
                                for h in range(HKV):
                                    nc.tensor.matmul(
                                        po[:, nn * 512 : (nn + 1) * 512],
                                        retrT3[:, h, mt * 128 : (mt + 1) * 128],
                                        wot3[:, h, nn * 512 : (nn + 1) * 512],
                                        start=(h == 0),
                                        stop=(h == HKV - 1),
                                    )
                            o_sb = osbp.tile([128, 1024], F32)
                            nc.scalar.activation(o_sb[:], po[:], AF.Copy)
                            nc.sync.dma_start(
                                out_d[mt * 128 : (mt + 1) * 128,
                                      ng * 1024 : (ng + 1) * 1024],
                                o_sb[:],
                            )

    nc.compile()
    return nc


def _get_nc():
    global _cached_nc
    if _cached_nc is None:
        _cached_nc = _build()
    return _cached_nc


def _prep_weights(Wq, bq, Wk, bk, Wv, bv, Wo, M, z):
    """Host-side weight repacking (layout only + Wo group-folding)."""
    Wq_sel = Wq.reshape(H, D, HID)[::G].reshape(HKV * D, HID)
    bq_sel = bq.reshape(H, D)[::G].reshape(HKV * D)
    W_cat = np.concatenate([Wq_sel, Wk, Wv], axis=0)          # [3072, HID]
    WT = np.ascontiguousarray(W_cat.T)                        # [HID, 3072]
    # wt[m, p, kt*128+j] = WT[kt*128+p, m*128+j]
    wt = np.ascontiguousarray(
        WT.reshape(KT, 128, MQKV, 128).transpose(2, 1, 0, 3)
    ).reshape(MQKV, 128, HID)

    Wo_eff = Wo.reshape(HID, HKV, G, D).sum(axis=2)           # [HID, HKV, D]
    # wot[ng, e, h*1024 + nslice] = Wo_eff[ng*1024+nslice, h, e]
    wot = np.ascontiguousarray(
        Wo_eff.reshape(4, 1024, HKV, D).transpose(0, 3, 2, 1)
    ).reshape(4, 128, HKV * NTOK)

    m_in = np.ascontiguousarray(M.transpose(1, 0, 2)).reshape(128, HKV * D)
    z_in = np.ascontiguousarray(z.T)                          # [128, HKV]
    b_cat = np.concatenate([bq_sel, bk, bv]).reshape(MQKV, 128)
    bias = np.ascontiguousarray(b_cat.T)                      # [128, MQKV]
    return wt, wot, m_in, z_in, bias


def kernel(hidden_states, Wq, bq, Wk, bk, Wv, bv, Wo, M, z):
    hidden_states = np.asarray(hidden_states, dtype=np.float32)
    Wq = np.asarray(Wq, dtype=np.float32)
    bq = np.asarray(bq, dtype=np.float32)
    Wk = np.asarray(Wk, dtype=np.float32)
    bk = np.asarray(bk, dtype=np.float32)
    Wv = np.asarray(Wv, dtype=np.float32)
    bv = np.asarray(bv, dtype=np.float32)
    Wo = np.asarray(Wo, dtype=np.float32)
    M = np.asarray(M, dtype=np.float32)
    z = np.asarray(z, dtype=np.float32)

    nc = _get_nc()
    wt, wot, m_in, z_in, bias = _prep_weights(Wq, bq, Wk, bk, Wv, bv, Wo, M, z)

    hs_flat = hidden_states.reshape(B * S, HID)
    in_maps = []
    for c in range(NCORES):
        in_maps.append({
            "hs": np.ascontiguousarray(hs_flat[c * NTOK : (c + 1) * NTOK]),
            "wt": wt, "wot": wot, "m_in": m_in, "z_in": z_in, "bias": bias,
        })

    global last_results
    res = bass_utils.run_bass_kernel_spmd(
        nc, in_maps, core_ids=list(range(NCORES)), trace=TRACE
    )
    last_results = res

    out = np.empty((B * S, HID), dtype=np.float32)
    dm_acc = np.zeros((128, HKV * D), dtype=np.float64)
    dz_acc = np.zeros((128, 16), dtype=np.float64)
    for c in range(NCORES):
        out[c * NTOK : (c + 1) * NTOK] = res.results[c]["out"]
        dm_acc += res.results[c]["dm"]
        dz_acc += res.results[c]["dz"]

    M_new = M + dm_acc.reshape(128, HKV, D).transpose(1, 0, 2).astype(np.float32)
    z_new = z + dz_acc.reshape(128, HKV, 2).sum(axis=2).T.astype(np.float32)
    return out.reshape(B, S, HID), M_new, z_new
